# revision 26
# baseline (speedup 1.0000x reference)
"""Trainium2 Bass kernel for DecGridDeepVPN (gnn_message_passing), 8-core SPMD.

Math (per batch row b, agents n=0..19):
  nsc[b]  = action_count[b].reshape(405) @ INFLOW            # [81]
  ir[b]   = sum_d min(nsc[b,d], demand[b,d])                 # scalar
  feat    = [nsc | onehot81(loc[b,n]) | la[b,n] | onehot20(n)]   # 187
  h0 = relu(feat @ W0); h1 = relu(h0 @ W1); out[b,n] = h1@W2 + b2 + ir[b]

Sharding: pure data-parallel over batch (4096 rows/core). Feature-major on
device: features on partitions, batch streaming on the free axis. MLP columns
are (b, g) pairs with g = n//4; agent classes j = n%4 occupy four 32-row
partition blocks.

v4 design (trace-driven):
  - streams in fp8e4m3 where precision allows (action_count, zg embedding,
    rhs0, and their weights); demand and MLP weights stay bf16; all PSUM
    accumulation fp32
  - 8 column blocks accumulate their W2 outputs into one PSUM bank at rows
    4q+j via sparse-M weight variants -> one [32,510] copy + one 32KB bf16
    DMA per 8 blocks (was shipping 87% zeros before)
  - vector h0 add+relu and scalar h1-relu operate on 2-block [128,1020]
    spans (PSUM banks are adjacent) to amortize per-op overhead
  - zb and ir are produced by one matmul pair into rows 0-32 of one bank and
    drained by a single scalar copy per b-block (ir rides in bf16)
  - input DMAs interleave action_count chunks with the first zg/rhs0 groups
    so phase M can start as soon as phase N's first b-block is reduced;
    zg/rhs0 ring is 4 groups deep; h0 matmuls of blocks 0-3 are emitted
    inside the phase-N instruction stream to hide DMA arrival stalls
  - K-padding rows (r0 rows 25-127, zb rows 33-127) are zeroed once on the
    (otherwise idle) gpsimd engine, off the critical path
"""

import dataclasses as _dc

import numpy as np
import ml_dtypes

import concourse.bass as bass
import concourse.mybir as mybir
from concourse.bass_utils import run_bass_kernel_spmd

BF16 = ml_dtypes.bfloat16
NP8 = ml_dtypes.float8_e4m3
F32 = mybir.dt.float32
BF = mybir.dt.bfloat16
F8 = mybir.dt.float8e4

S = 81          # grid states
GRID = 9
N_AG = 20       # agents
A = 5           # actions
B = 32768
NCORES = 8
BC = B // NCORES      # 4096 batch rows per core
NBB = 8               # b-blocks per core (phase N)
BB = BC // NBB        # 512
NG = BC * 5           # 20480 (b, g) columns per core
CB = 510              # col-block: multiple of 5, fits one PSUM bank
NCB = (NG + CB - 1) // CB   # 41 (last block = 80 cols)
NGRP = (NCB + 3) // 4       # 11 zg/rhs0 stream groups of <=4 blocks
GW = 4 * CB                 # 2040 cols per group stream
NGP = NGRP * GW             # 22440 padded column count
NP = (NCB + 1) // 2         # 21 block pairs
NT8 = (NCB + 7) // 8        # 6 output groups of <=8 blocks

# wpack (bf16 lhsT) column offsets
ZBL_OFF = 0               # [81, 128] W0a at cols 0..31
IRL_OFF = ZBL_OFF + 128   # ones column at col 32
WREP_OFF = IRL_OFF + 128  # rows 0..31: 4x identity
W1_OFF = WREP_OFF + 128
W0LE_OFF = W1_OFF + 128   # rows 0..24: w0le
W2_OFF = W0LE_OFF + 128   # 8 variants x 128 cols, rows live at 4q+j
NWCOL = W2_OFF + 8 * 128
# wp8 (fp8 lhsT) column offsets
WIN8_OFF = 0              # 4 x 81 inflow chunks
NW8COL = WIN8_OFF + 324


def _grid_inflow():
    moves = [(0, 0), (-1, 0), (1, 0), (0, -1), (0, 1)]
    inflow = np.zeros((S * A, S), np.float32)
    for s in range(S):
        r, c = divmod(s, GRID)
        for a, (dr, dc) in enumerate(moves):
            nr, nc_ = r + dr, c + dc
            d = nr * GRID + nc_ if (0 <= nr < GRID and 0 <= nc_ < GRID) else s
            inflow[s * A + a, d] = 1.0
    return inflow


def _cw(k):
    return min(CB, NG - k * CB)


def _build():
    nc = bass.Bass()

    acT = nc.declare_dram_parameter("acT", [4, 128, 4, 1024], F8, isOutput=False)
    demT = nc.declare_dram_parameter("demT", [S, BC], BF, isOutput=False)
    wpack = nc.declare_dram_parameter("wpack", [128, NWCOL], BF, isOutput=False)
    wp8 = nc.declare_dram_parameter("wp8", [128, NW8COL], F8, isOutput=False)
    zgb = nc.declare_dram_parameter("zgb", [128, NGP], F8, isOutput=False)
    rhs0b = nc.declare_dram_parameter("rhs0b", [32, NGP], BF, isOutput=False)
    out4 = nc.declare_dram_parameter("out4", [NT8, 32, 512], BF, isOutput=True)
    irO = nc.declare_dram_parameter("irO", [1, BC], BF, isOutput=True)

    from contextlib import ExitStack
    ctx = ExitStack()
    with ctx:
        s_ac = ctx.enter_context(nc.sbuf_tensor([128, 4 * BC], F8))
        s_dem = ctx.enter_context(nc.sbuf_tensor([S, BC], BF))
        s_nscT = ctx.enter_context(nc.sbuf_tensor([S, BC], BF))
        s_srv = ctx.enter_context(nc.sbuf_tensor([S, 2 * BB], BF))
        s_zb = ctx.enter_context(nc.sbuf_tensor([128, BC], BF))
        s_zgr = ctx.enter_context(nc.sbuf_tensor([128, 4 * GW], F8))
        s_r0r = ctx.enter_context(nc.sbuf_tensor([128, 4 * GW], BF))
        s_h0 = ctx.enter_context(nc.sbuf_tensor([128, 8 * CB], BF))
        s_h1 = ctx.enter_context(nc.sbuf_tensor([128, 8 * CB], BF))
        s_out = ctx.enter_context(nc.sbuf_tensor([32, NT8 * 512], BF))
        s_wp = ctx.enter_context(nc.sbuf_tensor([128, NWCOL], BF))
        s_wp8 = ctx.enter_context(nc.sbuf_tensor([128, NW8COL], F8))
        psum = ctx.enter_context(nc.psum_tensor([128, 4096], F32))

        d_w = ctx.enter_context(nc.semaphore())
        d_ac = [ctx.enter_context(nc.semaphore(f"d_ac{i}")) for i in range(4)]
        d_dem = ctx.enter_context(nc.semaphore())
        d_m = [ctx.enter_context(nc.semaphore(f"d_m{i}")) for i in range(4)]
        d_out = ctx.enter_context(nc.semaphore())
        t_nsc = ctx.enter_context(nc.semaphore())
        t_zbir = ctx.enter_context(nc.semaphore())
        t_h0 = ctx.enter_context(nc.semaphore())
        t_h1 = ctx.enter_context(nc.semaphore())
        t_o = ctx.enter_context(nc.semaphore())
        g_ms = ctx.enter_context(nc.semaphore())
        v_nsc = ctx.enter_context(nc.semaphore())
        v_min = ctx.enter_context(nc.semaphore())
        v_h0 = ctx.enter_context(nc.semaphore())
        sc_zbir = ctx.enter_context(nc.semaphore())
        sc_h1 = ctx.enter_context(nc.semaphore())
        sc_o = ctx.enter_context(nc.semaphore())
        block = ctx.enter_context(nc.Block())

        # psum bank map (bank = 512-f32 column chunk)
        # phase M: h0 banks 0-3 (k%4), h1 banks 4-5 (k%2), o banks 6-7 (t8%2)
        # phase N: zbir banks 4-5 (i%2), nsc banks 6-7 (i%2)
        def p_h0(k):
            return psum[:, (k % 4) * 512:(k % 4) * 512 + _cw(k)]

        def p_h0_pair(p):
            # blocks (2p, 2p+1): adjacent banks; 2D free AP [[512,2],[1,510]]
            base = ((2 * p) % 4) * 512
            ap = psum[:, base:base + 512]
            return _dc.replace(ap, ap=[ap.ap[0], [512, 2], [1, CB]])

        def p_h1(k):
            return psum[:, 2048 + (k % 2) * 512:2048 + (k % 2) * 512 + _cw(k)]

        def p_o(t8, cw):
            return psum[:, 3072 + (t8 % 2) * 512:3072 + (t8 % 2) * 512 + cw]

        def p_zbir(i):
            return psum[:, 2048 + (i % 2) * 512:2048 + (i % 2) * 512 + BB]

        def p_nsc(i):
            return psum[0:S, 3072 + (i % 2) * 512:3072 + (i % 2) * 512 + BB]

        def wp(off, ncol=128):
            return s_wp[0:128, off:off + ncol]

        def wp81(off, ncol=128):
            return s_wp[0:S, off:off + ncol]

        def zb_bcast(k):
            cw = _cw(k)
            nb = cw // 5
            b0 = k * (CB // 5)
            ap = s_zb[0:128, b0:b0 + nb]
            return _dc.replace(ap, ap=[ap.ap[0], [1, nb], [0, 5]])

        # zb copies (one per i-block) needed before block k's h0
        def f_zb(k):
            b_end = (k * CB + _cw(k)) // 5
            return (b_end + BB - 1) // BB

        def k_last(t):
            return min(4 * t + 3, NCB - 1)

        def pair2(sb, p, width):
            # two adjacent CB slots of an sbuf ring as one [*, width] span
            ap = sb[:, ((2 * p) % 8) * CB:((2 * p) % 8) * CB + width]
            return ap

        @block.sync
        def _(sync):
            def zg(t):
                if t == 0:
                    sync.wait_ge(g_ms, 2)  # padding memsets before r0 DMA
                if t >= 4:
                    sync.wait_ge(v_h0, 4 * (k_last(t - 4) // 2) + 2)
                sl = slice((t % 4) * GW, (t % 4) * GW + GW)
                sync.dma_start(s_zgr[:, sl], zgb[:, t * GW:(t + 1) * GW]
                               ).then_inc(d_m[t % 4], 16)
                sync.dma_start(s_r0r[0:32, sl], rhs0b[:, t * GW:(t + 1) * GW]
                               ).then_inc(d_m[t % 4], 16)

            def ac(ig):
                dst = s_ac[:, ig * 1024:ig * 1024 + 1024]
                dst = _dc.replace(dst, ap=[dst.ap[0], [4096, 4], [1, 1024]])
                sync.dma_start(dst, acT[ig]).then_inc(d_ac[ig], 16)

            sync.dma_start(s_wp[:, :], wpack[:, :]).then_inc(d_w, 16)
            sync.dma_start(s_wp8[:, :], wp8[:, :]).then_inc(d_w, 16)
            ac(0)
            ac(1)
            sync.dma_start(s_dem[:, :], demT[:, :]).then_inc(d_dem, 16)
            zg(0)
            ac(2)
            zg(1)
            ac(3)
            zg(2)
            zg(3)
            for t in range(4, NGRP):
                zg(t)
            sync.wait_ge(d_out, 16 * (NT8 + 1))

        @block.tensor
        def _(tensor):
            tensor.wait_ge(d_w, 32)

            def nsc(i):
                tensor.wait_ge(d_ac[i // 2], 16)
                if i >= 2:
                    tensor.wait_ge(v_nsc, i - 1)
                for c in range(4):
                    mm = nc.tensor.matmul(
                        p_nsc(i), s_wp8[0:128, WIN8_OFF + c * S:WIN8_OFF + (c + 1) * S],
                        s_ac[:, c * BC + i * BB:c * BC + (i + 1) * BB],
                        start=(c == 0), stop=(c == 3),
                    )
                mm.then_inc(t_nsc, 1)

            def zbir(i):
                tensor.wait_ge(v_nsc, i + 1)
                if i >= 2:
                    tensor.wait_ge(sc_zbir, i - 1)
                nc.tensor.matmul(
                    p_zbir(i), wp81(ZBL_OFF), s_nscT[:, i * BB:(i + 1) * BB],
                    start=True, stop=False,
                )
                tensor.wait_ge(v_min, i + 1)
                nc.tensor.matmul(
                    p_zbir(i), wp81(IRL_OFF),
                    s_srv[:, (i % 2) * BB:(i % 2 + 1) * BB],
                    start=False, stop=True,
                ).then_inc(t_zbir, 1)

            def h0a(k):
                t = k // 4
                q = k % 4
                cw = _cw(k)
                if k == 0:
                    tensor.wait_ge(g_ms, 2)  # r0/zb padding memsets
                tensor.wait_ge(d_m[t % 4], 32 * (t // 4 + 1))
                tensor.wait_ge(sc_zbir, f_zb(k))
                if k >= 4:
                    tensor.wait_ge(v_h0, 4 * ((k - 4) // 2) + 2)  # bank free
                nc.tensor.matmul(
                    p_h0(k), wp(W0LE_OFF),
                    s_r0r[0:128, (t % 4) * GW + q * CB:(t % 4) * GW + q * CB + cw],
                    start=True, stop=False, skip_group_check=True,
                )

            def h0b(k):
                nc.tensor.matmul(
                    p_h0(k), wp(WREP_OFF), zb_bcast(k),
                    start=False, stop=True, skip_group_check=True,
                ).then_inc(t_h0, 1)

            def h0(k):
                h0a(k)
                h0b(k)

            def h1(k):
                cw = _cw(k)
                tensor.wait_ge(v_h0, 4 * (k // 2) + 4)
                if k < 2:
                    tensor.wait_ge(sc_zbir, NBB)  # zbir copies off banks 4-5
                else:
                    tensor.wait_ge(sc_h1, k - 1)  # bank free (relu k-2 done)
                nc.tensor.matmul(
                    p_h1(k), wp(W1_OFF),
                    s_h0[:, (k % 8) * CB:(k % 8) * CB + cw],
                    start=True, stop=True,
                ).then_inc(t_h1, 1)

            def o(k):
                t8 = k // 8
                q8 = k % 8
                cw = _cw(k)
                tensor.wait_ge(sc_h1, k + 1)
                if q8 == 0 and t8 >= 2:
                    tensor.wait_ge(sc_o, t8 - 1)  # p_o bank free
                nc.tensor.matmul(
                    p_o(t8, cw), wp(W2_OFF + q8 * 128),
                    s_h1[:, (k % 8) * CB:(k % 8) * CB + cw],
                    start=(q8 == 0), stop=(q8 == 7 or k == NCB - 1),
                ).then_inc(t_o, 1)

            # phase N with h0(0..3) prefilled to hide ac DMA arrival stalls
            nsc(0)
            nsc(1)
            zbir(0)
            h0(0)
            h0(1)
            nsc(2)
            h0(2)
            h0(3)
            zbir(1)
            nsc(3)
            zbir(2)
            nsc(4)
            zbir(3)
            nsc(5)
            zbir(4)
            nsc(6)
            zbir(5)
            nsc(7)
            zbir(6)
            zbir(7)
            # phase M, pair-stepped: h0 pair | h1 pair (lag 2p) | o pair (lag 4p)
            def h0pair(P):
                ks = [k for k in (2 * P, 2 * P + 1) if k < NCB]
                for k in ks:
                    h0a(k)
                for k in ks:
                    h0b(k)

            def h1pair(P):
                for k in (2 * P, 2 * P + 1):
                    if k < NCB:
                        h1(k)

            def opair(P):
                for k in (2 * P, 2 * P + 1):
                    if k < NCB:
                        o(k)

            for PP in range(2, NP + 4):
                if PP < NP:
                    h0pair(PP)
                if 0 <= PP - 2 < NP:
                    h1pair(PP - 2)
                if 0 <= PP - 4 < NP:
                    opair(PP - 4)

        @block.vector
        def _(vector):
            I32 = mybir.dt.int32
            nc.vector.memset(s_zb[:, :].bitcast(I32), 0).then_inc(g_ms, 1)
            nc.vector.memset(s_r0r[:, :].bitcast(I32), 0).then_inc(g_ms, 1)
            for i in range(NBB):
                vector.wait_ge(t_nsc, i + 1)
                nc.vector.tensor_copy(
                    s_nscT[:, i * BB:(i + 1) * BB], p_nsc(i)
                ).then_inc(v_nsc, 1)
                if i == 0:
                    vector.wait_ge(d_dem, 16)
                if i >= 2:
                    vector.wait_ge(t_zbir, i - 1)  # s_srv slot free
                vector.wait_ge(v_nsc, i + 1)  # same-engine RAW
                nc.vector.tensor_tensor(
                    s_srv[:, (i % 2) * BB:(i % 2 + 1) * BB],
                    s_nscT[:, i * BB:(i + 1) * BB],
                    s_dem[:, i * BB:(i + 1) * BB],
                    mybir.AluOpType.min,
                ).then_inc(v_min, 1)
            for p in range(NP):
                k0 = 2 * p
                t = k0 // 4
                w = _cw(k0) + (_cw(k0 + 1) if k0 + 1 < NCB else 0)
                vector.wait_ge(t_h0, min(k0 + 2, NCB))
                if p >= 4:
                    vector.wait_ge(t_h1, 2 * p - 6)  # s_h0 slots free
                zsl = s_zgr[:, (t % 4) * GW + (k0 % 4) * CB:
                            (t % 4) * GW + (k0 % 4) * CB + w]
                if k0 + 1 < NCB:
                    pin = p_h0_pair(p)
                    out_ap = pair2(s_h0, p, w)
                    out_ap = _dc.replace(
                        out_ap, ap=[out_ap.ap[0], [CB, 2], [1, CB]])
                    zin = _dc.replace(zsl, ap=[zsl.ap[0], [CB, 2], [1, CB]])
                else:
                    pin = p_h0(k0)
                    out_ap = pair2(s_h0, p, w)
                    zin = zsl
                nc.vector.tensor_tensor(
                    out_ap, pin, zin, mybir.AluOpType.add
                ).then_inc(v_h0, 2)
                vector.wait_ge(v_h0, 4 * p + 2)  # same-engine RAW
                nc.vector.tensor_scalar_max(
                    pair2(s_h0, p, w), pair2(s_h0, p, w), 0.0
                ).then_inc(v_h0, 2)

        @block.scalar
        def _(scalar):
            AF = mybir.ActivationFunctionType
            scalar.wait_ge(g_ms, 1)  # s_zb padding memset before copies
            for i in range(NBB):
                scalar.wait_ge(t_zbir, i + 1)
                nc.scalar.copy(
                    s_zb[0:33, i * BB:(i + 1) * BB], p_zbir(i)[0:33, :]
                ).then_inc(sc_zbir, 1)
            scalar.wait_ge(sc_zbir, NBB)  # own copies retired (for DMA read)
            nc.scalar.dma_start(irO[:, :], s_zb[32:33, :]).then_inc(d_out, 16)

            def ocopy(t8c):
                scalar.wait_ge(t_o, min(8 * t8c + 8, NCB))
                cw8 = _cw(8 * t8c)
                nc.scalar.copy(
                    s_out[:, t8c * 512:t8c * 512 + cw8],
                    p_o(t8c, cw8)[0:32, :],
                ).then_inc(sc_o, 1)
                scalar.wait_ge(sc_o, t8c + 1)  # own copy retired (DMA read)
                nc.scalar.dma_start(
                    out4[t8c][:, 0:cw8], s_out[:, t8c * 512:t8c * 512 + cw8]
                ).then_inc(d_out, 16)

            for k in range(NCB):
                cw = _cw(k)
                scalar.wait_ge(t_h1, k + 1)
                if k >= 8:
                    scalar.wait_ge(t_o, k - 7)  # s_h1 slot free
                nc.scalar.activation(
                    s_h1[:, (k % 8) * CB:(k % 8) * CB + cw], p_h1(k), AF.Relu
                ).then_inc(sc_h1, 1)
                # drain o-group a few relus after its last o-matmul's h1 so
                # the copy never blocks the relus PE needs (deadlock-free)
                if k >= 11 and (k - 11) % 8 == 0:
                    ocopy((k - 11) // 8)
                elif k == 39:
                    ocopy(4)
                elif k == NCB - 1:
                    ocopy(5)

    return nc


_NC = {}


def _get_nc():
    if "v4" not in _NC:
        _NC["v4"] = _build()
    return _NC["v4"]


def _prep_core(obs, ac, la, zg_tab):
    """Host-side layout prep for one core's batch slice (all numpy)."""
    bc = obs.shape[0]
    out = {}
    # acT: [405, bc] padded to [512, bc] -> [ig, p, c, 1024] fp8
    acf = np.zeros((512, bc), np.float32)
    acf[:405] = ac.reshape(bc, 405).T
    out["acT"] = np.ascontiguousarray(
        acf.reshape(4, 128, 4, 1024).transpose(2, 1, 0, 3)
    ).astype(NP8)
    # demT: [81, bc] bf16
    out["demT"] = np.ascontiguousarray(obs[:, S:2 * S].T).astype(BF16)
    # zg: [128, NG] stacked by agent class, padded to NGP, fp8
    loc = obs[:, 2 * S:2 * S + N_AG].astype(np.int64)  # [bc, 20]
    zst = np.zeros((128, NGP), np.float32)
    for j in range(4):
        lj = loc[:, j::4].reshape(-1)            # cols (b, g)
        zst[32 * j:32 * j + 32, :bc * 5] = zg_tab[lj].T
    out["zgb"] = zst.astype(NP8)
    # rhs0: rows 0-19 la packed, rows 20-24 g-onehot, rows 25-31 zero pad
    r0 = np.zeros((32, NGP), np.float32)
    for j in range(4):
        r0[5 * j:5 * j + 5, :bc * 5] = (
            la[:, j::4, :].transpose(2, 0, 1).reshape(5, bc * 5)
        )
    r0[20:25, :bc * 5] = np.tile(np.eye(5, dtype=np.float32), (1, bc))
    out["rhs0b"] = r0.astype(BF16)
    return out


def _prep_weights(W0, W1, W2):
    W0a = W0[0:S]
    W0c, W0d = W0[2 * S:2 * S + A], W0[2 * S + A:]  # [5,32], [20,32]
    inflow = _grid_inflow()
    wpk = np.zeros((128, NWCOL), np.float32)
    wpk[0:S, ZBL_OFF:ZBL_OFF + 32] = W0a
    wpk[0:S, IRL_OFF + 32] = 1.0
    for j in range(4):
        wpk[0:32, WREP_OFF + 32 * j:WREP_OFF + 32 * j + 32] = np.eye(32)
        wpk[32 * j:32 * j + 32, W1_OFF + 32 * j:W1_OFF + 32 * j + 32] = W1
    for q in range(8):
        for j in range(4):
            wpk[32 * j:32 * j + 32, W2_OFF + 128 * q + 4 * q + j] = W2[:, 0]
    for j in range(4):
        wpk[5 * j:5 * j + 5, W0LE_OFF + 32 * j:W0LE_OFF + 32 * j + 32] = W0c
        for e in range(5):
            wpk[20 + e, W0LE_OFF + 32 * j:W0LE_OFF + 32 * j + 32] = W0d[4 * e + j]
    w8 = np.zeros((128, NW8COL), np.float32)
    infp = np.zeros((512, S), np.float32)
    infp[:405] = inflow
    for c in range(4):
        w8[:, WIN8_OFF + c * S:WIN8_OFF + (c + 1) * S] = infp[c * 128:(c + 1) * 128]
    return wpk.astype(BF16), w8.astype(NP8)


def _decode_core(r, b2):
    o = np.asarray(r["out4"], np.float32)[:, :, :CB]   # [t8, 32, 510]
    o = o.reshape(NT8, 8, 4, CB)                        # [t8, q8, j, c]
    o = o.transpose(0, 1, 3, 2).reshape(NT8 * 8 * CB, 4)[:NG]
    ob = o.reshape(BC, 5, 4).reshape(BC, N_AG)          # n = 4g + j
    return ob + np.asarray(r["irO"][0], np.float32)[:, None] + b2[0]


def kernel(obs, action_count, local_actions, W0, W1, W2, b2):
    obs = np.asarray(obs, np.float32)
    action_count = np.asarray(action_count, np.float32)
    local_actions = np.asarray(local_actions, np.float32)
    W0 = np.asarray(W0, np.float32)
    W1 = np.asarray(W1, np.float32)
    W2 = np.asarray(W2, np.float32)
    b2 = np.asarray(b2, np.float32)

    wpack, w8 = _prep_weights(W0, W1, W2)
    W0b = W0[S:2 * S]

    in_maps = []
    for c in range(NCORES):
        bsl = slice(c * BC, (c + 1) * BC)
        m = _prep_core(obs[bsl], action_count[bsl], local_actions[bsl], W0b)
        m["wpack"] = wpack
        m["wp8"] = w8
        in_maps.append(m)

    nc = _get_nc()
    res = run_bass_kernel_spmd(nc, in_maps, list(range(NCORES)))
    global LAST_RESULTS
    LAST_RESULTS = res

    out = np.empty((B, N_AG), np.float32)
    for c in range(NCORES):
        out[c * BC:(c + 1) * BC] = _decode_core(res.results[c], b2)
    return out
